# revision 45
# baseline (speedup 1.0000x reference)
"""Trainium2 Bass kernel for nn_CrossAttention (gnn_message_passing).

Math (per batch b):
  q = (q_w/sqrt(D)) @ pcd            (C, N)
  k = k_w @ neighbors                (C, N, K)
  v = v_w @ neighbors                (C, N, K)
  energy[h,n,k] = sum_d q[h*8+d,n] * k[h*8+d,n,k]
  attn = softmax_k(energy)           (exp without max-subtraction; |energy| is O(5))
  x[h*8+d, n] = sum_k attn[h,n,k] * v[h*8+d,n,k]

Mapping (per core, N sharded 8 ways -> NC=1024 points):
  - batches stacked in pairs on the 128 partitions: partition = (bb, c), bb in {0,1}
  - PE: block-diag weight stationaries for q/k/v projections (bf16 in, fp32 accum)
  - PE: block-ones stationary reduces over d AND replicates the result across d
    in one matmul -> energies land replicated, so exp's output is directly
    aligned with v for the attn*v multiply.
  - ACT: exp (PSUM -> SBUF bf16 eviction for free) + v eviction
  - DVE: q*k multiply, attn*v multiply, reciprocal, final normalization.
  - Pool (GPSIMD): softmax-denominator sum tree (SBUF-only data).
  - PE also sums attn*v over the K neighbors: K PSUM-accumulating identity
    matmuls per SB block, emitted in small chunks interleaved with later
    strips so the PE queue never head-blocks.

Scheduling: the two batch-pair passes are flattened into one continuous
64-strip stream. Per strip t the emission order is
  [nbh prefetch] [PE v,k matmuls; ACT v-evict; DVE q*k] [PE e-matmul(t-1) +
  xsum chunk; ACT exp(t-1)] [DVE attn*v, Pool den-tree for t-2]
so every cross-engine dependency has >= 1 strip of slack, and SB-block
finals are deferred one further block so DVE never waits on Pool. The
on-device repeat loop runs UNROLL bodies per For_i iteration to amortize
the loop's all-engine barrier + semaphore reset (~40us/iteration); bodies
overlap across the boundary via the tile rings.
"""

import numpy as np
import ml_dtypes

import concourse.bass as bass
import concourse.tile as tile
from concourse import bacc, mybir
from concourse.bass_utils import run_bass_kernel_spmd

BF16 = mybir.dt.bfloat16
F32 = mybir.dt.float32

B, CIN, N, K = 4, 64, 8192, 32
COUT, H = 64, 8
D = COUT // H
NCORES = 8
NC = N // NCORES  # points per core

_nc_cache = {}


def build_nc(NC=NC, S=32, R=1, attn_f32=False, psum_bufs=3, den_on_pool=True,
             HNC=256, x_via_pe=True, SB=256, den_block=False, UNROLL=4,
             den_via_pe=False, pool_mul_every=0):
    """Build the per-core Bass program.

    NC: points per core, S: strip size (points per DVE/ACT macro-op),
    R: on-device repeat count (for timing), attn_f32: keep attn in fp32,
    den_on_pool: denominator tree on GPSIMD, HNC: points per neighbor DMA,
    x_via_pe: sum attn*v over the K neighbors with PSUM-accumulating
    identity matmuls on the tensor engine instead of a DVE add tree,
    SB: block size (points) for the K-sum / finals batch.
    """
    key = (NC, S, R, attn_f32, psum_bufs, den_on_pool, HNC, x_via_pe, SB,
           den_block, UNROLL, den_via_pe, pool_mul_every)
    if key in _nc_cache:
        return _nc_cache[key]

    PTS = min(16, S)   # points per matmul chunk (16*32 = 512 cols = 1 PSUM bank)
    NCHUNK = S // PTS  # matmul chunks per strip
    assert S % PTS == 0 and NC % S == 0
    ADT = F32 if attn_f32 else BF16
    NS = NC // S       # strips per batch-pair pass
    TOT = 2 * NS
    SB = min(SB, NC)   # finals batch (points)
    assert SB % S == 0
    SPB = SB // S      # strips per SB block
    XCH = -(-K // SPB)  # xsum matmuls interleaved per strip
    QCH = min(512, NC)

    nc = bacc.Bacc("TRN2", target_bir_lowering=False, debug=False,
                   num_devices=NCORES)
    nbp = nc.dram_tensor("nbp", [2, 128, NC, K], BF16, kind="ExternalInput").ap()
    pcdp = nc.dram_tensor("pcdp", [2, 128, NC], BF16, kind="ExternalInput").ap()
    wq_d = nc.dram_tensor("wq", [128, 128], BF16, kind="ExternalInput").ap()
    wk_d = nc.dram_tensor("wk", [128, 128], BF16, kind="ExternalInput").ap()
    wv_d = nc.dram_tensor("wv", [128, 128], BF16, kind="ExternalInput").ap()
    wo_d = nc.dram_tensor("wones", [128, 128], BF16, kind="ExternalInput").ap()
    wi_d = nc.dram_tensor("wid", [128, 128], BF16, kind="ExternalInput").ap()
    xout = nc.dram_tensor("xout", [2, 128, NC], F32, kind="ExternalOutput").ap()

    with tile.TileContext(nc) as tc:
        with (
            tc.tile_pool(name="const", bufs=1) as cpool,
            tc.tile_pool(name="io", bufs=2) as iopool,
            tc.tile_pool(name="acc", bufs=2) as apool,
            tc.tile_pool(name="work", bufs=3) as wpool,
            tc.tile_pool(name="blk", bufs=2) as bpool,
            tc.tile_pool(name="ps", bufs=psum_bufs, space="PSUM") as pspool,
            tc.tile_pool(name="ps2", bufs=2, space="PSUM") as ps2pool,
        ):
            wq_t = cpool.tile([128, 128], BF16, tag="wq")
            nc.sync.dma_start(wq_t[:], wq_d[:])
            wk_t = cpool.tile([128, 128], BF16, tag="wk")
            nc.sync.dma_start(wk_t[:], wk_d[:])
            wv_t = cpool.tile([128, 128], BF16, tag="wv")
            nc.sync.dma_start(wv_t[:], wv_d[:])
            wo_t = cpool.tile([128, 128], BF16, tag="wo")
            nc.sync.dma_start(wo_t[:], wo_d[:])
            wi_t = cpool.tile([128, 128], BF16, tag="wi")
            nc.sync.dma_start(wi_t[:], wi_d[:])

            def body():
                q_sbs = {}
                x_strips = {}
                nbhs = {}
                n_halves = 2 * (NC // HNC)

                def load_half(h):
                    if h >= n_halves or h in nbhs:
                        return
                    hpp, off = divmod(h * HNC, NC)
                    t_ = iopool.tile([128, HNC, K], BF16, tag="nbh")
                    nc.sync.dma_start(t_[:], nbp[hpp, :, off:off + HNC, :])
                    nbhs[h] = t_

                def pp_head(pp):
                    pcd_t = iopool.tile([128, NC], BF16, tag="pcd")
                    nc.sync.dma_start(pcd_t[:], pcdp[pp])
                    q_sb = apool.tile([128, NC], F32, tag="q")
                    for h in range(NC // QCH):
                        q_ps = pspool.tile([128, S, K], F32, tag="ps")
                        qp = q_ps[:, 0:QCH // K, :]
                        nc.tensor.matmul(qp, wq_t[:],
                                         pcd_t[:, h * QCH:(h + 1) * QCH],
                                         start=True, stop=True)
                        nc.vector.tensor_copy(q_sb[:, h * QCH:(h + 1) * QCH], qp)
                    q_sbs[pp] = q_sb
                    x_strips[pp] = apool.tile([128, NC], F32, tag="xs",
                                              name="xs")

                def front(t):
                    pp, s = divmod(t, NS)
                    n0 = s * S
                    if s == 0:
                        pp_head(pp)
                        if t == 0:
                            load_half(0)
                    if n0 % HNC == 0:
                        load_half((pp * NC + n0) // HNC + 1)  # prefetch next
                    nbh = nbhs[(pp * NC + n0) // HNC]
                    nb_t = nbh[:, n0 % HNC:n0 % HNC + S, :]
                    v_ps = pspool.tile([128, S, K], F32, tag="ps")
                    for j in range(NCHUNK):
                        nc.tensor.matmul(
                            v_ps[:, j * PTS:(j + 1) * PTS, :], wv_t[:],
                            nb_t[:, j * PTS:(j + 1) * PTS, :],
                            start=True, stop=True)
                    v_sb = wpool.tile([128, S, K], ADT, tag="vsb")
                    nc.scalar.copy(v_sb[:], v_ps[:])
                    k_ps = pspool.tile([128, S, K], F32, tag="ps")
                    for j in range(NCHUNK):
                        nc.tensor.matmul(
                            k_ps[:, j * PTS:(j + 1) * PTS, :], wk_t[:],
                            nb_t[:, j * PTS:(j + 1) * PTS, :],
                            start=True, stop=True)
                    prod = wpool.tile([128, S, K], BF16, tag="prod")
                    qb = (q_sbs[pp][:, n0:n0 + S].unsqueeze(2)
                          .broadcast_to([128, S, K]))
                    nc.vector.tensor_mul(prod[:], k_ps[:], qb)
                    return (pp, n0, prod, v_sb)

                blocks = {}  # (pp, block) -> (attn_b, xacc, td4)
                xsum_st = [None]  # pending chunked xsum: [x_ps, xacc, next_k]

                def emit_xsum_chunk():
                    st = xsum_st[0]
                    if st is None:
                        return
                    x_ps, mv, k0 = st
                    for k in range(k0, min(k0 + XCH, K)):
                        if den_via_pe:
                            out, src = x_ps[:, :, :], mv[:, :, :, k]
                        else:
                            out, src = x_ps[:, :], mv[:, :, k]
                        nc.tensor.matmul(out, wi_t[:], src,
                                         start=(k == 0), stop=(k == K - 1),
                                         skip_group_check=True)
                    st[2] = k0 + XCH
                    if st[2] >= K:
                        xsum_st[0] = None

                def mid(st):
                    pp, n0, prod, v_sb = st
                    e_ps = pspool.tile([128, S, K], F32, tag="ps")
                    for j in range(NCHUNK):
                        nc.tensor.matmul(
                            e_ps[:, j * PTS:(j + 1) * PTS, :], wo_t[:],
                            prod[:, j * PTS:(j + 1) * PTS, :],
                            start=True, stop=True)
                    emit_xsum_chunk()
                    o = n0 % SB
                    bkey = (pp, n0 // SB)
                    if o == 0:
                        if den_via_pe:
                            # one tile holds both K-sum operands: [:,0] the
                            # attn*v products, [:,1] the attn weights, so one
                            # matmul per k reduces both into PSUM
                            ab = bpool.tile([128, 2, SB, K], ADT, tag="ab",
                                            name="ab")
                            attn_b, xacc, td4 = ab[:, 1], ab[:, 0], ab
                        else:
                            attn_b = bpool.tile([128, SB, K], ADT, tag="attn",
                                                name="attn")
                            if x_via_pe:
                                xacc = bpool.tile([128, SB, K], ADT, tag="pb",
                                                  name="pb")
                            else:
                                xacc = wpool.tile([128, SB, 4], ADT, tag="tx4",
                                                  name="tx4")
                            td4 = wpool.tile([128, SB, 4], ADT, tag="td4",
                                             name="td4")
                        blocks[bkey] = (attn_b, xacc, td4)
                    attn_b, xacc, td4 = blocks[bkey]
                    nc.scalar.activation(attn_b[:, o:o + S, :], e_ps[:],
                                         mybir.ActivationFunctionType.Exp)
                    return (pp, n0, v_sb, attn_b, xacc, td4)

                def tree(src_t, t4, eng, o):
                    cur = src_t
                    w = 16
                    while w >= 8:
                        nxt = wpool.tile([128, S, w], ADT,
                                         tag=f"s{t4.name[:3]}{w}")
                        eng.tensor_add(nxt[:], cur[:, :, 0:w],
                                       cur[:, :, w:2 * w])
                        cur = nxt
                        w //= 2
                    eng.tensor_add(t4[:, o:o + S, :],
                                   cur[:, :, 0:4], cur[:, :, 4:8])

                def back(st):
                    pp, n0, v_sb, attn_full, xacc, td4 = st
                    o = n0 % SB
                    attn = attn_full[:, o:o + S, :]
                    if x_via_pe:
                        # attn*v lands in the block-wide product tile; the
                        # K-sum happens later on the PE (identity matmuls).
                        # Optionally every Nth strip's multiply runs on the
                        # (otherwise idle) Pool engine.
                        mul_eng = nc.vector
                        if pool_mul_every and (n0 // S) % pool_mul_every == (
                                pool_mul_every - 1):
                            mul_eng = nc.gpsimd
                        mul_eng.tensor_mul(xacc[:, o:o + S, :], v_sb[:], attn)
                    else:
                        prod2 = wpool.tile([128, S, K], ADT, tag="prod2")
                        nc.vector.tensor_mul(prod2[:], v_sb[:], attn)
                        tree(prod2, xacc, nc.vector, o)
                    if not den_block and not den_via_pe:
                        tree(attn, td4, nc.gpsimd if den_on_pool else nc.vector,
                             o)
                    if o + S == SB:
                        return (pp, n0 + S - SB, xacc, td4, attn_full)
                    return None

                def den_pool(fin):
                    # whole-block denominator tree on GPSIMD: 3 launches per
                    # block instead of 3 per strip
                    pp, nb0, xacc, td4, attn_b = fin
                    d16 = bpool.tile([128, SB, 16], ADT, tag="d16", name="d16")
                    nc.gpsimd.tensor_add(d16[:], attn_b[:, :, 0:16],
                                         attn_b[:, :, 16:32])
                    d8 = bpool.tile([128, SB, 8], ADT, tag="d8", name="d8")
                    nc.gpsimd.tensor_add(d8[:], d16[:, :, 0:8], d16[:, :, 8:16])
                    nc.gpsimd.tensor_add(td4[:], d8[:, :, 0:4], d8[:, :, 4:8])

                def xsum_pe(fin):
                    # x_un[:, n] = sum_k prod2[:, n, k] via K PSUM-accumulated
                    # identity matmuls (one k-slice per pass), emitted in
                    # chunks interleaved with the next block's strips so the
                    # PE queue never gets a head-blocking burst.
                    pp, nb0, xacc, td4 = fin[0], fin[1], fin[2], fin[3]
                    while xsum_st[0] is not None:  # drain any previous block
                        emit_xsum_chunk()
                    if den_via_pe:
                        x_ps = ps2pool.tile([128, 2, SB], F32, tag="xps",
                                            name="xps")
                        xsum_st[0] = [x_ps, td4, 0]  # td4 slot holds ab
                    else:
                        x_ps = ps2pool.tile([128, SB], F32, tag="xps")
                        xsum_st[0] = [x_ps, xacc, 0]
                    return x_ps

                def finals(fin, x_ps):
                    pp, nb0, xacc, td4 = fin[0], fin[1], fin[2], fin[3]
                    if den_via_pe:
                        den = x_ps[:, 1, :]
                    else:
                        t2 = wpool.tile([128, SB, 2], ADT, tag="f2td4")
                        nc.vector.tensor_add(t2[:], td4[:, :, 0:2],
                                             td4[:, :, 2:4])
                        den = wpool.tile([128, SB], F32, tag="f1td4")
                        nc.vector.tensor_add(den[:], t2[:, :, 0], t2[:, :, 1])
                    if den_via_pe:
                        x_un = x_ps[:, 0, :]
                    elif x_via_pe:
                        x_un = x_ps
                    else:
                        t2x = wpool.tile([128, SB, 2], ADT, tag="f2tx4")
                        nc.vector.tensor_add(t2x[:], xacc[:, :, 0:2],
                                             xacc[:, :, 2:4])
                        x_un = wpool.tile([128, SB], F32, tag="f1tx4")
                        nc.vector.tensor_add(x_un[:], t2x[:, :, 0], t2x[:, :, 1])
                    rden = wpool.tile([128, SB], F32, tag="rden")
                    nc.vector.reciprocal(rden[:], den[:])
                    nc.vector.tensor_mul(x_strips[pp][:, nb0:nb0 + SB],
                                         x_un[:], rden[:])
                    if nb0 + SB == NC:
                        nc.sync.dma_start(xout[pp], x_strips[pp][:])

                pend_fin = [None]

                def handle_fin(fin):
                    if fin is None:
                        return
                    if den_block:
                        den_pool(fin)
                    x_ps = xsum_pe(fin) if x_via_pe else None
                    if pend_fin[0] is not None:
                        finals(*pend_fin[0])
                        pend_fin[0] = None
                    pp, nb0 = fin[0], fin[1]
                    if nb0 + SB == NC:
                        while xsum_st[0] is not None:  # flush remaining chunks
                            emit_xsum_chunk()
                        finals(fin, x_ps)  # last block of the pass: flush now
                    else:
                        pend_fin[0] = (fin, x_ps)

                pend_mid = None
                pend_back = None
                for t in range(TOT):
                    f = front(t)
                    if pend_mid is not None:
                        b = mid(pend_mid)
                        if pend_back is not None:
                            handle_fin(back(pend_back))
                        pend_back = b
                    pend_mid = f
                b = mid(pend_mid)
                handle_fin(back(pend_back))
                handle_fin(back(b))

            if R <= 8:
                for _ in range(R):
                    body()
            else:
                # amortize For_i's per-iteration all-engine barrier +
                # semaphore reset across UNROLL bodies
                assert R % UNROLL == 0, f"R={R} not divisible by {UNROLL}"
                with tc.For_i(0, R // UNROLL, 1):
                    for _ in range(UNROLL):
                        body()

    nc.compile()
    _nc_cache[key] = nc
    return nc


def prep_inputs(pcd, neighbors, q_w, k_w, v_w, NC=NC):
    """Host-side prep: cast to bf16, pair-stack batches, build stationaries."""
    bf = ml_dtypes.bfloat16
    s = 1.0 / np.sqrt(np.float32(D))
    qwT = (q_w.astype(np.float32) * s).T.astype(bf)  # (c, hd)
    kwT = k_w.T.astype(bf)
    vwT = v_w.T.astype(bf)

    def blockdiag(m):
        z = np.zeros((128, 128), dtype=bf)
        z[:64, :64] = m
        z[64:, 64:] = m
        return z

    wq = blockdiag(qwT)
    wk = blockdiag(kwT)
    wv = blockdiag(vwT)
    blk = np.kron(np.eye(H, dtype=np.float32), np.ones((D, D), np.float32))
    wones = blockdiag(blk.astype(bf))
    wid = np.eye(128, dtype=bf)

    nbs = neighbors.reshape(2, 2 * CIN, N, K)    # (pair, bb*64+c, n, k)
    pcds = pcd.reshape(2, 2 * CIN, N)
    ncores = N // NC
    in_maps = []
    for i in range(ncores):
        sl = slice(i * NC, (i + 1) * NC)
        in_maps.append({
            "nbp": np.ascontiguousarray(nbs[:, :, sl, :]).astype(bf),
            "pcdp": np.ascontiguousarray(pcds[:, :, sl]).astype(bf),
            "wq": wq, "wk": wk, "wv": wv, "wones": wones, "wid": wid,
        })
    return in_maps


def assemble_output(results, NC=NC):
    ncores = len(results)
    out = np.empty((B, COUT, N), dtype=np.float32)
    for i, r in enumerate(results):
        x = r["xout"].reshape(B, COUT, NC)  # (2,128,NC) -> (4,64,NC)
        out[:, :, i * NC:(i + 1) * NC] = x
    return out


BEST = dict(S=32, psum_bufs=3, UNROLL=12)


def kernel(pcd, neighbors, q_w, k_w, v_w):
    pcd = np.asarray(pcd, dtype=np.float32)
    neighbors = np.asarray(neighbors, dtype=np.float32)
    nc = build_nc(NC=NC, R=1, **BEST)
    in_maps = prep_inputs(pcd, neighbors, q_w, k_w, v_w)
    res = run_bass_kernel_spmd(nc, in_maps, core_ids=list(range(NCORES)))
    return assemble_output(res.results)


if __name__ == "__main__":
    rng = np.random.default_rng(0)
    ins = {
        "pcd": rng.standard_normal((B, CIN, N), dtype=np.float32),
        "neighbors": rng.standard_normal((B, CIN, N, K), dtype=np.float32),
        "q_w": (rng.standard_normal((COUT, CIN), dtype=np.float32) / 8.0),
        "k_w": (rng.standard_normal((COUT, CIN), dtype=np.float32) / 8.0),
        "v_w": (rng.standard_normal((COUT, CIN), dtype=np.float32) / 8.0),
    }
    out = kernel(**ins)
    print("kernel output", out.shape, out.dtype)
